# revision 1
# baseline (speedup 1.0000x reference)
"""GCN (4x GCNConv + global_add_pool + MLP) on 8 Trainium2 NeuronCores.

Sharding: nodes are partitioned into 8 contiguous blocks of 12544 (dst
partitioning).  Each edge is owned by the core that owns its dst node, so the
scatter-add is core-local; the gather side reads from a replicated per-layer
table built with one AllGather per layer.  The per-edge normalisation
dinv[src]*dinv[dst] is split: dinv[src] is folded into the table rows,
dinv[dst] into the one-hot scatter matrix values.
"""

import math

import numpy as np

P = 128          # partitions / edge-tile size / node-tile size
NFEAT = 32
HID = 96
NG = 2048        # graphs
NCORES = 8
NPC = 12544      # nodes per core (98 * 128)
NT = NPC // P    # 98 node tiles per core
NPAD = NPC * NCORES
NG_PAD = NG + 512   # pool scatter buffer rows (>= gbase_max + GT*128)


# ----------------------------------------------------------------------------
# Host-side preprocessing: edge bucketing / sorting / tiling, norm factors.
# ----------------------------------------------------------------------------

def _prep(x, edge_index, batch):
    N = x.shape[0]
    src = np.asarray(edge_index[0], dtype=np.int64)
    dst = np.asarray(edge_index[1], dtype=np.int64)
    loops = np.arange(N, dtype=np.int64)
    src = np.concatenate([src, loops])
    dst = np.concatenate([dst, loops])
    deg = np.bincount(dst, minlength=N).astype(np.float64)
    dinv = np.where(deg > 0, 1.0 / np.sqrt(np.maximum(deg, 1.0)), 0.0).astype(
        np.float32
    )
    batch = np.asarray(batch, dtype=np.int64)

    owner = dst // NPC
    percore = []
    for c in range(NCORES):
        m = owner == c
        s_c = src[m]
        d_c = dst[m] - c * NPC
        dv_c = dinv[dst[m]]
        o = np.argsort(d_c, kind="stable")
        s_c, d_c, dv_c = s_c[o], d_c[o], dv_c[o]
        t_c = d_c // P
        cnt = np.bincount(t_c, minlength=NT)
        percore.append((s_c, d_c, dv_c, t_c, cnt))

    M = max(
        1, max(int(math.ceil(int(cnt.max()) / P)) for (_, _, _, _, cnt) in percore)
    )
    EPT = M * P

    # graph-tile count (same for all cores; one-hot auto-zeros out of range)
    GT = 1
    for c in range(NCORES):
        n0 = c * NPC
        nreal = min(NPC, max(0, N - n0))
        if nreal > 0:
            gb = int(batch[n0])
            gmax = int(batch[n0 + nreal - 1])
            GT = max(GT, int(math.ceil((gmax - gb + 1) / P)))
    assert NG + GT * P <= NG_PAD + P  # scatter rows stay in-bounds

    inputs = []
    for c in range(NCORES):
        s_c, d_c, dv_c, t_c, cnt = percore[c]
        srcg = np.zeros((NT, EPT), np.int32)
        doff = np.full((NT, EPT), -1.0, np.float32)  # -1 never matches iota
        dval = np.zeros((NT, EPT), np.float32)
        start = np.zeros(NT + 1, np.int64)
        start[1:] = np.cumsum(cnt)
        slot = np.arange(len(d_c)) - start[t_c]
        flat = t_c * EPT + slot
        srcg.reshape(-1)[flat] = s_c.astype(np.int32)
        doff.reshape(-1)[flat] = (d_c - t_c * P).astype(np.float32)
        dval.reshape(-1)[flat] = dv_c

        def dev(a):  # [NT, M*P] -> [P, NT*M]  (slot j = m*P + p)
            return np.ascontiguousarray(
                a.reshape(NT, M, P).transpose(2, 0, 1).reshape(P, NT * M)
            )

        n0 = c * NPC
        nreal = min(NPC, max(0, N - n0))
        dloc = np.zeros(NPC, np.float32)
        dloc[:nreal] = dinv[n0 : n0 + nreal]
        xs = np.zeros((NPC, NFEAT), np.float32)
        xs[:nreal] = np.asarray(x, np.float32)[n0 : n0 + nreal]
        gbase = int(batch[n0]) if nreal > 0 else 0
        pg = np.full(NPC, 1.0e9, np.float32)
        pg[:nreal] = (batch[n0 : n0 + nreal] - gbase).astype(np.float32)
        # scatter row indices for pooling: row = gbase + g*128 + p
        growidx = (
            gbase
            + np.arange(GT, dtype=np.int32)[None, :] * P
            + np.arange(P, dtype=np.int32)[:, None]
        ).astype(np.int32)

        inputs.append(
            dict(
                xin=xs,
                srcg=dev(srcg),
                doff=dev(doff),
                dval=dev(dval),
                dinv=np.ascontiguousarray(dloc.reshape(NT, P).T),
                poolg=np.ascontiguousarray(pg.reshape(NT, P).T),
                growidx=growidx,
            )
        )
    return inputs, M, GT


# ----------------------------------------------------------------------------
# Numpy emulation of the device program (for debugging the index plumbing).
# ----------------------------------------------------------------------------

def _emulate(inputs, M, GT, weights):
    outs = []
    tables = [None] * NCORES
    h = [inp["xin"].copy() for inp in inputs]
    Ws = [weights["W1"], weights["W2"], weights["W3"], weights["W4"]]
    bs = [weights["b1"], weights["b2"], weights["b3"], weights["b4"]]
    for l in range(4):
        for c in range(NCORES):
            xw = h[c] @ Ws[l]
            tables[c] = xw * inputs[c]["dinv"].T.reshape(NPC, 1)
        table = np.concatenate(tables, axis=0)  # [NPAD, HID]
        for c in range(NCORES):
            inp = inputs[c]
            hn = np.zeros((NPC, HID), np.float32)
            for t in range(NT):
                acc = np.zeros((P, HID), np.float32)
                for m in range(M):
                    k = t * M + m
                    rows = table[inp["srcg"][:, k]]  # [P, HID]
                    iota = np.arange(P, dtype=np.float32)
                    oh = (
                        (iota[None, :] == inp["doff"][:, k : k + 1])
                        * inp["dval"][:, k : k + 1]
                    ).astype(np.float32)
                    acc += oh.T @ rows
                hn[t * P : (t + 1) * P] = np.maximum(acc + bs[l][None, :], 0.0)
            h[c] = hn
    # pool
    gsum = np.zeros((NG_PAD, HID), np.float32)
    for c in range(NCORES):
        inp = inputs[c]
        for g in range(GT):
            acc = np.zeros((P, HID), np.float32)
            for t in range(NT):
                iota = g * P + np.arange(P, dtype=np.float32)
                oh = (
                    iota[None, :] == inp["poolg"][:, t : t + 1]
                ).astype(np.float32)  # [p, graph]
                acc += oh.T @ h[c][t * P : (t + 1) * P]
            gsum[inp["growidx"][:, g]] += acc
    g = gsum[:NG]
    z = np.maximum(g @ weights["Wf1"] + weights["bf1"][None, :], 0.0)
    return z @ weights["Wf2"] + weights["bf2"]


# ----------------------------------------------------------------------------
# Bass program.
# ----------------------------------------------------------------------------

def _build_program(M, GT, bf2val):
    from concourse import bacc, bass, mybir, tile

    f32 = mybir.dt.float32
    i32 = mybir.dt.int32
    AF = mybir.ActivationFunctionType
    OP = mybir.AluOpType

    nc = bacc.Bacc("TRN2", target_bir_lowering=False, debug=False)

    x_p = nc.declare_dram_parameter("xin", [NPC, NFEAT], f32, isOutput=False)
    srcg_p = nc.declare_dram_parameter("srcg", [P, NT * M], i32, isOutput=False)
    doff_p = nc.declare_dram_parameter("doff", [P, NT * M], f32, isOutput=False)
    dval_p = nc.declare_dram_parameter("dval", [P, NT * M], f32, isOutput=False)
    dinv_p = nc.declare_dram_parameter("dinv", [P, NT], f32, isOutput=False)
    poolg_p = nc.declare_dram_parameter("poolg", [P, NT], f32, isOutput=False)
    grow_p = nc.declare_dram_parameter("growidx", [P, GT], i32, isOutput=False)
    w_ps = [
        nc.declare_dram_parameter("w1", [NFEAT, HID], f32, isOutput=False),
        nc.declare_dram_parameter("w2", [HID, HID], f32, isOutput=False),
        nc.declare_dram_parameter("w3", [HID, HID], f32, isOutput=False),
        nc.declare_dram_parameter("w4", [HID, HID], f32, isOutput=False),
    ]
    b_ps = [
        nc.declare_dram_parameter(f"b{l + 1}", [1, HID], f32, isOutput=False)
        for l in range(4)
    ]
    wf1_p = nc.declare_dram_parameter("wf1", [HID, 32], f32, isOutput=False)
    bf1_p = nc.declare_dram_parameter("bf1", [32, 1], f32, isOutput=False)
    wf2_p = nc.declare_dram_parameter("wf2", [32, 1], f32, isOutput=False)
    out_p = nc.declare_dram_parameter("out", [1, NG], f32, isOutput=True)

    groups = [list(range(NCORES))]

    with tile.TileContext(nc) as tc:
        with (
            tc.tile_pool(name="const", bufs=1) as cp,
            tc.tile_pool(name="sb", bufs=1) as sb,
            tc.tile_pool(name="ps", bufs=2, space="PSUM") as ps,
            tc.tile_pool(name="dram", bufs=1, space="DRAM") as dp,
        ):
            # ---- persistent SBUF tensors -------------------------------
            h = cp.tile([P, NT, HID], f32)
            x_sb = cp.tile([P, NT, NFEAT], f32)
            srcg_sb = cp.tile([P, NT * M], i32)
            doff_sb = cp.tile([P, NT * M], f32)
            dval_sb = cp.tile([P, NT * M], f32)
            dinv_sb = cp.tile([P, NT], f32)
            poolg_sb = cp.tile([P, NT], f32)
            grow_sb = cp.tile([P, GT], i32)
            iota_i = cp.tile([P, GT * P], i32)
            iota_f = cp.tile([P, GT * P], f32)
            ident = cp.tile([P, P], f32)
            ones1 = cp.tile([1, P], f32)
            zero_sb = cp.tile([P, HID], f32)
            w_sb = [
                cp.tile([NFEAT if l == 0 else HID, HID], f32, name=f"w{l}sb")
                for l in range(4)
            ]
            brow = [cp.tile([1, HID], f32, name=f"brow{l}") for l in range(4)]
            bias_sb = [cp.tile([P, HID], f32, name=f"bias{l}") for l in range(4)]
            wf1_sb = cp.tile([HID, 32], f32)
            bf1_sb = cp.tile([32, 1], f32)
            wf2_sb = cp.tile([32, 1], f32)
            gsumT = cp.tile([HID, NG], f32)
            zT = cp.tile([32, NG], f32)
            osb = cp.tile([1, NG], f32)

            # ---- DRAM scratch ------------------------------------------
            contrib = [
                dp.tile([NPC, HID], f32, name=f"contrib{l}", bufs=1)
                for l in range(4)
            ]
            table = [
                dp.tile([NPAD, HID], f32, name=f"table{l}", bufs=1, addr_space="Shared")
                for l in range(4)
            ]
            gin = dp.tile([NG_PAD, HID], f32)
            gout = dp.tile([NG_PAD, HID], f32, addr_space="Shared")

            # ---- load constants ----------------------------------------
            nc.sync.dma_start(
                out=x_sb[:], in_=x_p[:].rearrange("(t p) f -> p t f", p=P)
            )
            nc.sync.dma_start(out=srcg_sb[:], in_=srcg_p[:])
            nc.sync.dma_start(out=doff_sb[:], in_=doff_p[:])
            nc.sync.dma_start(out=dval_sb[:], in_=dval_p[:])
            nc.sync.dma_start(out=dinv_sb[:], in_=dinv_p[:])
            nc.sync.dma_start(out=poolg_sb[:], in_=poolg_p[:])
            nc.sync.dma_start(out=grow_sb[:], in_=grow_p[:])
            for l in range(4):
                nc.sync.dma_start(out=w_sb[l][:], in_=w_ps[l][:])
                nc.sync.dma_start(out=brow[l][:], in_=b_ps[l][:])
            nc.sync.dma_start(out=wf1_sb[:], in_=wf1_p[:])
            nc.sync.dma_start(out=bf1_sb[:], in_=bf1_p[:])
            nc.sync.dma_start(out=wf2_sb[:], in_=wf2_p[:])

            from concourse.masks import make_identity

            make_identity(nc, ident[:])
            nc.gpsimd.iota(
                iota_i[:], pattern=[[1, GT * P]], base=0, channel_multiplier=0
            )
            nc.vector.tensor_copy(out=iota_f[:], in_=iota_i[:])
            nc.vector.memset(ones1[:], 1.0)
            nc.vector.memset(zero_sb[:], 0.0)
            nc.vector.memset(osb[:], 0.0)

            # broadcast biases to [P, HID] via outer product with ones
            for l in range(4):
                pb = ps.tile([P, HID], f32, tag="xw")
                nc.tensor.matmul(
                    out=pb[:], lhsT=ones1[:], rhs=brow[l][:], start=True, stop=True
                )
                nc.any.tensor_copy(out=bias_sb[l][:], in_=pb[:])

            # zero the pool scatter buffer
            for r in range(NG_PAD // P):
                nc.sync.dma_start(
                    out=gin[r * P : (r + 1) * P, :], in_=zero_sb[:]
                )

            # ---- 4 GCN layers ------------------------------------------
            for l in range(4):
                K = NFEAT if l == 0 else HID
                # table build: contrib rows = dinv[n] * (h @ W)
                for t in range(NT):
                    hsrc = x_sb[:, t, :] if l == 0 else h[:, t, :]
                    pst = ps.tile([K, P], f32, tag="trans")
                    nc.tensor.transpose(out=pst[:], in_=hsrc, identity=ident[:])
                    hT = sb.tile([K, P], f32, tag="hT", bufs=3)
                    nc.any.tensor_copy(out=hT[:], in_=pst[:])
                    pxw = ps.tile([P, HID], f32, tag="xw")
                    nc.tensor.matmul(
                        out=pxw[:], lhsT=hT[:], rhs=w_sb[l][:], start=True, stop=True
                    )
                    xws = sb.tile([P, HID], f32, tag="xws", bufs=3)
                    nc.vector.tensor_scalar(
                        out=xws[:],
                        in0=pxw[:],
                        scalar1=dinv_sb[:, t : t + 1],
                        scalar2=None,
                        op0=OP.mult,
                    )
                    nc.sync.dma_start(
                        out=contrib[l][t * P : (t + 1) * P, :], in_=xws[:]
                    )
                nc.gpsimd.collective_compute(
                    "AllGather",
                    OP.bypass,
                    replica_groups=groups,
                    ins=[contrib[l][:]],
                    outs=[table[l][:]],
                )
                # message pass: gather rows, one-hot matmul scatter
                for t in range(NT):
                    msg = sb.tile([P, M * HID], f32, tag="msg", bufs=3)
                    nc.gpsimd.indirect_dma_start(
                        out=msg[:],
                        out_offset=None,
                        in_=table[l][:],
                        in_offset=bass.IndirectOffsetOnAxis(
                            ap=srcg_sb[:, t * M : (t + 1) * M], axis=0
                        ),
                    )
                    oh = sb.tile([P, M * P], f32, tag="oh", bufs=3)
                    for m in range(M):
                        nc.vector.tensor_scalar(
                            out=oh[:, m * P : (m + 1) * P],
                            in0=iota_f[:, 0:P],
                            scalar1=doff_sb[:, t * M + m : t * M + m + 1],
                            scalar2=dval_sb[:, t * M + m : t * M + m + 1],
                            op0=OP.is_equal,
                            op1=OP.mult,
                        )
                    pacc = ps.tile([P, HID], f32, tag="acc")
                    for m in range(M):
                        nc.tensor.matmul(
                            out=pacc[:],
                            lhsT=oh[:, m * P : (m + 1) * P],
                            rhs=msg[:, m * HID : (m + 1) * HID],
                            start=(m == 0),
                            stop=(m == M - 1),
                        )
                    nc.vector.tensor_add(
                        out=h[:, t, :], in0=pacc[:], in1=bias_sb[l][:]
                    )
                    nc.scalar.activation(
                        out=h[:, t, :], in_=h[:, t, :], func=AF.Relu
                    )

            # ---- global_add_pool ---------------------------------------
            for g in range(GT):
                pg_ps = ps.tile([P, HID], f32, tag="acc")
                for t in range(NT):
                    ohp = sb.tile([P, P], f32, tag="ohp", bufs=4)
                    nc.vector.tensor_scalar(
                        out=ohp[:],
                        in0=iota_f[:, g * P : (g + 1) * P],
                        scalar1=poolg_sb[:, t : t + 1],
                        scalar2=None,
                        op0=OP.is_equal,
                    )
                    nc.tensor.matmul(
                        out=pg_ps[:],
                        lhsT=ohp[:],
                        rhs=h[:, t, :],
                        start=(t == 0),
                        stop=(t == NT - 1),
                    )
                gsb = sb.tile([P, HID], f32, tag="gsb", bufs=2)
                nc.any.tensor_copy(out=gsb[:], in_=pg_ps[:])
                nc.gpsimd.indirect_dma_start(
                    out=gin[:],
                    out_offset=bass.IndirectOffsetOnAxis(
                        ap=grow_sb[:, g : g + 1], axis=0
                    ),
                    in_=gsb[:],
                    in_offset=None,
                )

            nc.gpsimd.collective_compute(
                "AllReduce",
                OP.add,
                replica_groups=groups,
                ins=[gin[:]],
                outs=[gout[:]],
            )

            # transpose g back: [NG, HID] -> [HID, NG]
            for j in range(NG // P):
                grow_t = sb.tile([P, HID], f32, tag="gsb", bufs=2)
                nc.sync.dma_start(
                    out=grow_t[:], in_=gout[j * P : (j + 1) * P, :]
                )
                pT = ps.tile([HID, P], f32, tag="trans")
                nc.tensor.transpose(out=pT[:], in_=grow_t[:], identity=ident[:])
                nc.any.tensor_copy(out=gsumT[:, j * P : (j + 1) * P], in_=pT[:])

            # ---- MLP head ----------------------------------------------
            for j in range(NG // 512):
                pz = ps.tile([32, 512], f32, tag="xw")
                nc.tensor.matmul(
                    out=pz[:],
                    lhsT=wf1_sb[:],
                    rhs=gsumT[:, j * 512 : (j + 1) * 512],
                    start=True,
                    stop=True,
                )
                nc.scalar.activation(
                    out=zT[:, j * 512 : (j + 1) * 512],
                    in_=pz[:],
                    func=AF.Relu,
                    bias=bf1_sb[:, 0:1],
                    scale=1.0,
                )
                po = ps.tile([1, 512], f32, tag="trans")
                nc.tensor.matmul(
                    out=po[:],
                    lhsT=wf2_sb[:],
                    rhs=zT[:, j * 512 : (j + 1) * 512],
                    start=True,
                    stop=True,
                )
                nc.vector.tensor_scalar(
                    out=osb[:, j * 512 : (j + 1) * 512],
                    in0=po[:],
                    scalar1=float(bf2val),
                    scalar2=None,
                    op0=OP.add,
                )
            nc.sync.dma_start(out=out_p[:], in_=osb[:])

    nc.finalize()
    return nc


# ----------------------------------------------------------------------------
# Entry point.
# ----------------------------------------------------------------------------

_RUN_KWARGS = {}


def kernel(
    x,
    edge_index,
    batch,
    W1,
    b1,
    W2,
    b2,
    W3,
    b3,
    W4,
    b4,
    Wf1,
    bf1,
    Wf2,
    bf2,
):
    from concourse.bass_utils import run_bass_kernel_spmd

    inputs, M, GT = _prep(np.asarray(x), np.asarray(edge_index), np.asarray(batch))
    bf2val = float(np.asarray(bf2).reshape(-1)[0])
    nc = _build_program(M, GT, bf2val)

    shared = dict(
        w1=np.asarray(W1, np.float32),
        w2=np.asarray(W2, np.float32),
        w3=np.asarray(W3, np.float32),
        w4=np.asarray(W4, np.float32),
        b1=np.asarray(b1, np.float32).reshape(1, HID),
        b2=np.asarray(b2, np.float32).reshape(1, HID),
        b3=np.asarray(b3, np.float32).reshape(1, HID),
        b4=np.asarray(b4, np.float32).reshape(1, HID),
        wf1=np.asarray(Wf1, np.float32),
        bf1=np.asarray(bf1, np.float32).reshape(32, 1),
        wf2=np.asarray(Wf2, np.float32).reshape(32, 1),
    )
    in_maps = [{**inputs[c], **shared} for c in range(NCORES)]
    res = run_bass_kernel_spmd(
        nc, in_maps, core_ids=list(range(NCORES)), **_RUN_KWARGS
    )
    global _LAST_RES
    _LAST_RES = res
    out = np.asarray(res.results[0]["out"]).reshape(NG, 1).astype(np.float32)
    return out


_LAST_RES = None

